# revision 18
# baseline (speedup 1.0000x reference)
"""Causal self-attention on 8 Trainium2 NeuronCores.

Problem: B=2, T=2048, C=1024, 16 heads x 64 dim, fp32.

Sharding: tensor-parallel over heads x data-parallel over batch.
Each core owns one batch element (cores 0-3 -> b=0, 4-7 -> b=1) and a
group of 4 consecutive heads. Each core computes:
  - QKV projection for its 4 heads (producing qT/kT transposed, V natural)
  - causal attention for its 4 heads (scores kept transposed: ST[tk, tq])
  - partial output projection (its heads' rows of w_proj)
The host sums the 4 partial projections per batch and adds b_proj.

All inputs/outputs bf16 (halves DMA; matmuls run bf16 = full PE rate).

Fused pipeline: the program is one stream per slab s of 512 queries:
  QKV(0); for s: ATT(s, pair0), ATT(s, pair1) with QKV(s+1)/OP(s-1)
  matmul units interleaved as PE fillers inside the tk loop, so the PE
  stays busy while ACT computes exp(ST).
Softmax denominators come from a ones-column appended to V (row 64 of
the PV psum accumulator).  Slab-end drain: DVE reciprocal directly from
PSUM, broadcast 1/d across 64 partitions with a tiny PE matmul
(ones x rec -> psum), one DVE multiply produces normalized bf16 yT.
Mask multiplies for diagonal tiles run on GpSimd (otherwise idle).

Device layouts (per core, DRAM):
  xT   [1024, 2048] bf16  x[b] transposed (channels on partitions)
  wqk  [1024, 512]  bf16  cols: q(h0)|q(h1)|k(h0)|k(h1)|q(h2)|q(h3)|k(h2)|k(h3)
  wv   [1024, 256]  bf16  v cols of the 4 heads
  wo   [256, 1024]  bf16  w_proj rows of the 4 heads
  bqk  [4, 128]     f32   rows: pair0-q, pair0-k, pair1-q, pair1-k biases
  bv   [256]        f32   v bias of the 4 heads
  mask [128, 128]   bf16  mask[i,j] = 1 if i<=j else 0 (tk<=tq keep)
  out  [2048, 1024] bf16  partial (pre-bias) output projection
"""

import numpy as np

B, T, C = 2, 2048, 1024
NH, DH = 16, 64
NCORES = 8
HPC = 4  # heads per core
P = 128
CK = C // P  # 8 contraction tiles over channels
NT = T // P  # 16 token tiles
SLAB = 512
NSL = T // SLAB  # 4 tq slabs

_CACHE = {}


def _build_program():
    from contextlib import ExitStack

    import concourse.bacc as bacc
    import concourse.bass as bass
    import concourse.tile as tile
    from concourse import mybir

    f32 = mybir.dt.float32
    f32r = mybir.dt.float32r
    bf16 = mybir.dt.bfloat16
    AF = mybir.ActivationFunctionType

    nc = bacc.Bacc(
        "TRN2", target_bir_lowering=False, debug=False, num_devices=NCORES
    )

    xT = nc.dram_tensor("xT", [C, T], bf16, kind="ExternalInput").ap()
    wqk = nc.dram_tensor("wqk", [C, 4 * P], bf16, kind="ExternalInput").ap()
    wv = nc.dram_tensor("wv", [C, HPC * DH], bf16, kind="ExternalInput").ap()
    wo = nc.dram_tensor("wo", [HPC * DH, C], bf16, kind="ExternalInput").ap()
    bqk = nc.dram_tensor("bqk", [4, P], f32, kind="ExternalInput").ap()
    bv = nc.dram_tensor("bv", [HPC * DH], f32, kind="ExternalInput").ap()
    mask = nc.dram_tensor("mask", [P, P], bf16, kind="ExternalInput").ap()
    out = nc.dram_tensor("out", [T, C], bf16, kind="ExternalOutput").ap()

    def r(ap):
        return ap.bitcast(f32r)

    with tile.TileContext(nc) as tc, ExitStack() as ctx:
        const = ctx.enter_context(tc.tile_pool(name="const", bufs=1))
        # PSUM (8 banks of [128,512]f32): pp 2x2 + psy 2x1 + misc 2x1
        ppp = ctx.enter_context(tc.tile_pool(name="ppp", bufs=2, space="PSUM"))
        psyp = ctx.enter_context(tc.tile_pool(name="psyp", bufs=2, space="PSUM"))
        miscp = ctx.enter_context(tc.tile_pool(name="miscp", bufs=2, space="PSUM"))
        expp = ctx.enter_context(tc.tile_pool(name="expp", bufs=3))
        recp = ctx.enter_context(tc.tile_pool(name="recp", bufs=3))
        outp = ctx.enter_context(tc.tile_pool(name="outp", bufs=2))

        x_sb = [
            const.tile([P, CK, SLAB], bf16, name=f"x{s}") for s in range(NSL)
        ]
        wqk_sb = const.tile([P, CK, 4 * P], bf16, name="wqk_sb")
        wv_sb = const.tile([P, CK, HPC * DH], bf16, name="wv_sb")
        wo_sb = const.tile([P, 2, C], bf16, name="wo_sb")
        bqk_sb = const.tile([P, 4], f32, name="bqk_sb")
        bv_sb = const.tile([P, HPC, DH], f32, name="bv_sb")
        mask_sb = const.tile([P, P], bf16, name="mask_sb")
        ones_sb = const.tile([1, DH], bf16, name="ones_sb")
        v_sb = const.tile([P, NT, HPC, DH + 1], bf16, name="v_sb")
        qT = [const.tile([P, T], bf16, name=f"qT{p}") for p in range(2)]
        kT = [const.tile([P, T], bf16, name=f"kT{p}") for p in range(2)]
        yT = [const.tile([P, T], bf16, name=f"yT{p}") for p in range(2)]

        # --- loads: few big DMAs (issue cost ~0.6us each), priority order ---
        wqkv_ = wqk.rearrange("(k p) n -> p k n", p=P)
        xTv = xT.rearrange("(k p) t -> p k t", p=P)
        h = CK // 2
        nc.sync.dma_start(out=wqk_sb[:, :h, :], in_=wqkv_[:, :h, :])
        nc.sync.dma_start(out=x_sb[0][:, :h, :], in_=xTv[:, :h, 0:SLAB])
        nc.sync.dma_start(out=wqk_sb[:, h:, :], in_=wqkv_[:, h:, :])
        nc.sync.dma_start(out=x_sb[0][:, h:, :], in_=xTv[:, h:, 0:SLAB])
        nc.sync.dma_start(out=bqk_sb[:], in_=bqk.rearrange("r p -> p r"))
        nc.sync.dma_start(out=mask_sb[:], in_=mask)
        bv_bcast = bass.AP(
            tensor=bv.tensor,
            offset=bv.offset,
            ap=[[0, P], *bv.rearrange("(h d) -> h d", d=DH).ap],
        )
        nc.sync.dma_start(out=bv_sb[:], in_=bv_bcast)
        nc.sync.dma_start(out=wv_sb[:], in_=wv.rearrange("(k p) n -> p k n", p=P))
        for s in range(1, NSL):
            nc.sync.dma_start(
                out=x_sb[s][:], in_=xTv[:, :, s * SLAB : (s + 1) * SLAB]
            )
        nc.sync.dma_start(out=wo_sb[:], in_=wo.rearrange("(r p) n -> p r n", p=P))
        nc.vector.memset(v_sb[:, :, :, DH : DH + 1], 1.0)
        nc.vector.memset(ones_sb[:], 1.0)

        # --- work units ---
        def qk_block(s, blk):
            """One q/k column block of QKV(s): 8 chained MMs + ACT bias."""
            p, qk = divmod(blk, 2)
            dst = qT[p] if qk == 0 else kT[p]
            ps = miscp.tile([P, SLAB], f32, name="ps_qkv", tag="m")
            for k in range(CK):
                nc.tensor.matmul(
                    ps[:],
                    lhsT=wqk_sb[:, k, blk * P : (blk + 1) * P],
                    rhs=x_sb[s][:, k, :],
                    start=(k == 0),
                    stop=(k == CK - 1),
                )
            nc.vector.tensor_scalar_add(
                out=dst[:, s * SLAB : (s + 1) * SLAB],
                in0=ps[:],
                scalar1=bqk_sb[:, blk : blk + 1],
            )

        def v_block(s, tt):
            """V for token tile 4s+tt: 8 chained MMs + DVE bias add."""
            t = 4 * s + tt
            ps = miscp.tile([P, SLAB], f32, name="ps_v", tag="m")
            for k in range(CK):
                nc.tensor.matmul(
                    ps[:, : HPC * DH],
                    lhsT=x_sb[s][:, k, tt * P : (tt + 1) * P],
                    rhs=wv_sb[:, k, :],
                    start=(k == 0),
                    stop=(k == CK - 1),
                )
            nc.vector.tensor_add(
                out=v_sb[:, t, :, 0:DH],
                in0=ps[:, : HPC * DH].rearrange("p (h d) -> p h d", d=DH),
                in1=bv_sb[:],
            )

        def op_unit(t):
            """Output projection for token tile t + DVE drain + DMA."""
            ob = outp.tile([P, C], bf16, name="ob", tag="ob")
            for ns in range(2):
                ps = miscp.tile([P, SLAB], f32, name="pso", tag="m")
                for p in range(2):
                    nc.tensor.matmul(
                        ps[:],
                        lhsT=yT[p][:, t * P : (t + 1) * P],
                        rhs=wo_sb[:, p, ns * SLAB : (ns + 1) * SLAB],
                        start=(p == 0),
                        stop=(p == 1),
                    )
                nc.vector.tensor_copy(
                    out=ob[:, ns * SLAB : (ns + 1) * SLAB], in_=ps[:]
                )
                nc.sync.dma_start(
                    out=out[t * P : (t + 1) * P, ns * SLAB : (ns + 1) * SLAB],
                    in_=ob[:, ns * SLAB : (ns + 1) * SLAB],
                )

        def att(s, p, fillers):
            """Causal attention for head pair p over tq slab s.

            Pops one PE filler unit per tk step to cover exp latency.
            """
            ntk = 4 * s + 4
            psy = [
                psyp.tile([P, SLAB], f32, name=f"psy{hp}", tag="psy")
                for hp in range(2)
            ]

            def off_of(tk):
                d = tk - 4 * s
                return d * P if d >= 0 else 0

            pend = {}
            exd = {}

            def st(tk):
                off = off_of(tk)
                pp = ppp.tile([P, 2 * SLAB], f32, name="pp", tag="pp")
                for hp in range(2):
                    nc.tensor.matmul(
                        pp[:, hp * SLAB + off : (hp + 1) * SLAB],
                        lhsT=kT[p][hp * DH : (hp + 1) * DH, tk * P : (tk + 1) * P],
                        rhs=qT[p][hp * DH : (hp + 1) * DH, s * SLAB + off : (s + 1) * SLAB],
                        start=True,
                        stop=True,
                    )
                pend[tk] = pp

            def do_exp(tk):
                off = off_of(tk)
                pp = pend.pop(tk)
                ex = expp.tile([P, 2 * SLAB], bf16, name="ex", tag="ex")
                ppv = pp[:].rearrange("q (h n) -> q h n", h=2)[:, :, off:]
                exv = ex[:].rearrange("q (h n) -> q h n", h=2)[:, :, off:]
                nc.scalar.activation(
                    out=exv,
                    in_=ppv,
                    func=AF.Exp,
                    scale=float(1.0 / np.sqrt(DH)),
                )
                if tk - 4 * s >= 0:
                    for hp in range(2):
                        nc.vector.tensor_mul(
                            out=ex[:, hp * SLAB + off : hp * SLAB + off + P],
                            in0=ex[:, hp * SLAB + off : hp * SLAB + off + P],
                            in1=mask_sb[:],
                        )
                exd[tk] = ex

            st(0)
            if ntk > 1:
                st(1)
            do_exp(0)
            for tk in range(ntk):
                off = off_of(tk)
                if tk + 2 < ntk:
                    st(tk + 2)
                if fillers:
                    fillers.pop(0)()
                if tk + 1 < ntk:
                    do_exp(tk + 1)
                ex = exd.pop(tk)
                for hp in range(2):
                    nc.tensor.matmul(
                        psy[hp][0 : DH + 1, off:],
                        lhsT=v_sb[:, tk, 2 * p + hp, :],
                        rhs=ex[:, hp * SLAB + off : (hp + 1) * SLAB],
                        start=(tk == 0),
                        stop=(tk == ntk - 1),
                    )
            # drain: ONE copy [65,512] frees psy ~0.7us after the last PV;
            # 1/d, broadcast, and the normalizing mul then run lazily off
            # the critical path (yT is only needed by OP a slab later).
            yraws, sms = [], []
            for hp in range(2):
                yraw = recp.tile([DH, SLAB], f32, name="yraw", tag="yraw")
                nc.vector.tensor_copy(out=yraw[:], in_=psy[hp][0:DH, :])
                yraws.append(yraw)
                sm = recp.tile([1, SLAB], f32, name="sm", tag="sm")
                nc.vector.tensor_copy(out=sm[:], in_=psy[hp][DH : DH + 1, :])
                sms.append(sm)
            recs = []
            for hp in range(2):
                rec = recp.tile([1, SLAB], f32, name="rec", tag="rec")
                nc.vector.reciprocal_approx_fast(out=rec[:], in_=sms[hp][:])
                recs.append(rec)
            rbs = []
            for hp in range(2):
                rb = recp.tile([DH, SLAB], f32, name="rb", tag="rb")
                nc.gpsimd.partition_broadcast(out_ap=rb[:], in_ap=recs[hp][:])
                rbs.append(rb)
            for hp in range(2):
                nc.vector.tensor_mul(
                    out=yT[p][hp * DH : (hp + 1) * DH, s * SLAB : (s + 1) * SLAB],
                    in0=yraws[hp][0:DH, :],
                    in1=rbs[hp][:],
                )

        # --- fused schedule ---
        for blk in range(4):
            qk_block(0, blk)
        for tt in range(4):
            v_block(0, tt)
        # OP token tiles are spread over later slabs (OP first in each
        # filler list so their DVE drains clear before the slab-end chain).
        op_sched = {1: [0, 1], 2: [2, 3, 4, 5], 3: [6, 7, 8, 9, 10, 11]}
        for s in range(NSL):
            fillers = [(lambda t=t: op_unit(t)) for t in op_sched.get(s, [])]
            if s + 1 < NSL:
                fillers += [
                    (lambda b=b, ss=s + 1: qk_block(ss, b)) for b in range(4)
                ]
                fillers += [
                    (lambda t=t, ss=s + 1: v_block(ss, t)) for t in range(4)
                ]
            att(s, 0, fillers)
            att(s, 1, fillers)
            for f in fillers:
                f()
        for t in range(4 * (NSL - 1), NT):
            op_unit(t)

    nc.compile()
    return nc


def get_program():
    if "nc" not in _CACHE:
        _CACHE["nc"] = _build_program()
    return _CACHE["nc"]


def make_core_inputs(x, w_attn, b_attn, w_proj, core):
    """Host-side shard preparation for one core."""
    import ml_dtypes

    bf16 = ml_dtypes.bfloat16
    b = core // 4
    g = core % 4
    heads = [4 * g + i for i in range(HPC)]

    xT = np.ascontiguousarray(np.asarray(x[b], np.float32).T.astype(bf16))

    def qcols(h):
        return w_attn[:, h * DH : (h + 1) * DH]

    def kcols(h):
        return w_attn[:, C + h * DH : C + (h + 1) * DH]

    def vcols(h):
        return w_attn[:, 2 * C + h * DH : 2 * C + (h + 1) * DH]

    h0, h1, h2, h3 = heads
    wqk = np.ascontiguousarray(
        np.concatenate(
            [qcols(h0), qcols(h1), kcols(h0), kcols(h1),
             qcols(h2), qcols(h3), kcols(h2), kcols(h3)],
            axis=1,
        ).astype(bf16)
    )
    wv = np.ascontiguousarray(
        np.concatenate([vcols(h) for h in heads], axis=1).astype(bf16)
    )
    bqk = np.stack(
        [
            np.concatenate([b_attn[h0 * DH : (h0 + 1) * DH], b_attn[h1 * DH : (h1 + 1) * DH]]),
            np.concatenate([b_attn[C + h0 * DH : C + (h0 + 1) * DH], b_attn[C + h1 * DH : C + (h1 + 1) * DH]]),
            np.concatenate([b_attn[h2 * DH : (h2 + 1) * DH], b_attn[h3 * DH : (h3 + 1) * DH]]),
            np.concatenate([b_attn[C + h2 * DH : C + (h2 + 1) * DH], b_attn[C + h3 * DH : C + (h3 + 1) * DH]]),
        ]
    ).astype(np.float32)
    bv = np.concatenate(
        [b_attn[2 * C + h * DH : 2 * C + (h + 1) * DH] for h in heads]
    ).astype(np.float32)
    wo = np.ascontiguousarray(
        w_proj[heads[0] * DH : (heads[-1] + 1) * DH, :].astype(bf16)
    )
    mask = np.triu(np.ones((P, P))).astype(bf16)
    return {
        "xT": xT,
        "wqk": wqk,
        "wv": wv,
        "wo": wo,
        "bqk": np.ascontiguousarray(bqk),
        "bv": np.ascontiguousarray(bv),
        "mask": mask,
    }


def kernel(x, w_attn, b_attn, w_proj, b_proj):
    from concourse.bass_utils import run_bass_kernel_spmd

    x = np.asarray(x, np.float32)
    w_attn = np.asarray(w_attn, np.float32)
    b_attn = np.asarray(b_attn, np.float32)
    w_proj = np.asarray(w_proj, np.float32)
    b_proj = np.asarray(b_proj, np.float32)

    nc = get_program()
    in_maps = [
        make_core_inputs(x, w_attn, b_attn, w_proj, core) for core in range(NCORES)
    ]
    res = run_bass_kernel_spmd(nc, in_maps, core_ids=list(range(NCORES)))
    outs = [np.asarray(m["out"], np.float32) for m in res.results]

    y = np.empty((B, T, C), np.float32)
    for b in range(B):
        y[b] = outs[4 * b] + outs[4 * b + 1] + outs[4 * b + 2] + outs[4 * b + 3]
        y[b] += b_proj[None, :]
    return y
